# revision 1
# baseline (speedup 1.0000x reference)
"""BiMambaBlock kernel for 8 TRN2 NeuronCores (Bass/Tile via PJRT).

Sharding: 8 cores = (modality i, direction dir, batch b) — each core runs the
full per-sequence pipeline on one channel-shuffled (and, for dir=1, L-flipped)
sequence x_i[b] of shape (96, 9216):
  LayerNorm -> RMSNorm -> fused causal-conv+input-projection (TensorE) -> SiLU
  -> B/C/dt projections -> selective scan (DVE tensor_tensor_scan over
  (d,n)-partition tiles, chunked along L with carried state) ->
  y = (scan + xc*Dp) * silu(z) -> 0.5 * output projection (+ residual on the
  fwd core). Host sums fwd/bwd partials and reshapes.

Self-contained: only needs numpy + jax + the concourse stack at
/opt/trn_rl_repo (present in the execution container).
"""
import sys
for _p in ("/opt/trn_rl_repo",):
    if _p not in sys.path:
        sys.path.insert(0, _p)
import numpy as np
from contextlib import ExitStack

import concourse.bass as bass
import concourse.bacc as bacc
import concourse.tile as tile
from concourse import mybir

F32 = mybir.dt.float32
F32R = mybir.dt.float32r
AF = mybir.ActivationFunctionType
OP = mybir.AluOpType

C, DI, N, R, K = 96, 192, 16, 6, 4
HH = WW = 96
L_FULL = HH * WW     # 9216
EPS = 1e-5

TC = 768             # time chunk
SUB = 384            # psum sub-chunk

# all PE matmuls run plain fp32 (fp32r needs producer-side rounding, which
# the BIR verifier enforces); the hot B/C broadcasts instead use an exact
# bf16 selector x (hi+lo bf16) split accumulated in PSUM (~1.5e-5 rel).
MM_HEAD_R = False
MM_DBL_R = False
MM_DT_R = False
MM_YACC_R = False
MM_OUT_R = False
BF16 = mybir.dt.bfloat16


def _mm(nc, out, lhsT, rhs, r, **kw):
    if r:
        lhsT = lhsT.bitcast(F32R)
        rhs = rhs.bitcast(F32R)
    nc.tensor.matmul(out, lhsT, rhs, **kw)


def build_program(L=L_FULL, Tc=TC):
    NCHUNK = L // Tc
    NSUB = Tc // SUB
    TC3 = Tc + 3
    nc = bacc.Bacc("TRN2", target_bir_lowering=False, debug=False)

    x_in = nc.dram_tensor("x", [C, L], F32, kind="ExternalInput")
    wIN = nc.dram_tensor("wIN", [C, K * DI], F32, kind="ExternalInput")
    wZ = nc.dram_tensor("wZ", [C, DI], F32, kind="ExternalInput")
    wXP = nc.dram_tensor("wXP", [C, 2 * 38], F32, kind="ExternalInput")
    wDT = nc.dram_tensor("wDT", [R, DI], F32, kind="ExternalInput")
    wA = nc.dram_tensor("wA", [C, 2 * N], F32, kind="ExternalInput")
    wOUT = nc.dram_tensor("wOUT", [C, 2 * C], F32, kind="ExternalInput")
    vec2 = nc.dram_tensor("vec2", [C, 8], F32, kind="ExternalInput")
    vec1 = nc.dram_tensor("vec1", [C, 3], F32, kind="ExternalInput")
    gate_in = nc.dram_tensor("gate", [1, 1], F32, kind="ExternalInput")
    eye_in = nc.dram_tensor("eye", [C, C], F32, kind="ExternalInput")
    sel_in = nc.dram_tensor("sel", [N, N * C], BF16, kind="ExternalInput")

    p_out = nc.dram_tensor("p", [C, L], F32, kind="ExternalOutput")

    with ExitStack() as ctx:
        tc = ctx.enter_context(tile.TileContext(nc))
        wp = ctx.enter_context(tc.tile_pool(name="wts", bufs=1))
        px = ctx.enter_context(tc.tile_pool(name="px", bufs=2))
        ph = ctx.enter_context(tc.tile_pool(name="ph", bufs=2))
        pt0 = ctx.enter_context(tc.tile_pool(name="pt0", bufs=1))
        pt1 = ctx.enter_context(tc.tile_pool(name="pt1", bufs=2))
        psp = ctx.enter_context(tc.tile_pool(name="psp", bufs=1))
        psmall = ctx.enter_context(tc.tile_pool(name="psmall", bufs=1))
        pxc = ctx.enter_context(tc.tile_pool(name="pxc", bufs=2))
        pg = ctx.enter_context(tc.tile_pool(name="pg", bufs=1))
        pdbl = ctx.enter_context(tc.tile_pool(name="pdbl", bufs=2))
        pdt = ctx.enter_context(tc.tile_pool(name="pdt", bufs=2))
        pdtx = ctx.enter_context(tc.tile_pool(name="pdtx", bufs=2))
        psc = ctx.enter_context(tc.tile_pool(name="psc", bufs=2))
        phh = ctx.enter_context(tc.tile_pool(name="phh", bufs=2))
        phc = ctx.enter_context(tc.tile_pool(name="phc", bufs=2))
        pst = ctx.enter_context(tc.tile_pool(name="pst", bufs=1))
        ptail = ctx.enter_context(tc.tile_pool(name="ptail", bufs=1))
        ppr = ctx.enter_context(tc.tile_pool(name="ppr", bufs=1))

        qh = ctx.enter_context(tc.tile_pool(name="qh", bufs=2, space="PSUM"))
        qbc = ctx.enter_context(tc.tile_pool(name="qbc", bufs=2, space="PSUM"))
        qy = ctx.enter_context(tc.tile_pool(name="qy", bufs=1, space="PSUM"))

        w_in = wp.tile([C, K * DI], F32); nc.sync.dma_start(w_in[:], wIN[:])
        w_z = wp.tile([C, DI], F32); nc.sync.dma_start(w_z[:], wZ[:])
        w_xp = wp.tile([C, 2 * 38], F32); nc.sync.dma_start(w_xp[:], wXP[:])
        w_dt = wp.tile([R, DI], F32); nc.sync.dma_start(w_dt[:], wDT[:])
        w_a = wp.tile([C, 2 * N], F32); nc.sync.dma_start(w_a[:], wA[:])
        w_out = wp.tile([C, 2 * C], F32); nc.sync.dma_start(w_out[:], wOUT[:])
        v2 = wp.tile([C, 8], F32); nc.sync.dma_start(v2[:], vec2[:])
        v1 = wp.tile([C, 3], F32); nc.sync.dma_start(v1[:], vec1[:])
        gt = wp.tile([1, 1], F32); nc.sync.dma_start(gt[:], gate_in[:])
        eye = wp.tile([C, C], F32); nc.sync.dma_start(eye[:], eye_in[:])
        sel = wp.tile([N, N * C], BF16); nc.sync.dma_start(sel[:], sel_in[:])
        ones_col = wp.tile([C, 1], F32); nc.vector.memset(ones_col[:], 1.0)
        epsc = wp.tile([1, 1], F32); nc.vector.memset(epsc[:], EPS)
        ones_row = wp.tile([1, C], F32); nc.vector.memset(ones_row[:], 1.0)
        gate_c = wp.tile([C, 1], F32)
        qg = qh.tile([C, 1], F32, tag="a")
        _mm(nc, qg[:], ones_row[:], gt[:], False)
        nc.scalar.copy(gate_c[:], qg[:])

        # probes: absorb cross-engine waits (TensorScalarPtr ops: 1 wait slot)
        prv = ppr.tile([1, 8], F32)
        pra = ppr.tile([1, 8], F32)
        nc.vector.tensor_copy(prv[:, 0:1], v1[:1, 0:1])
        nc.vector.tensor_copy(prv[:, 1:2], v2[:1, 0:1])
        nc.vector.tensor_copy(prv[:, 2:3], gate_c[:1, 0:1])
        nc.scalar.copy(pra[:, 0:1], w_a[:1, 0:1])
        nc.scalar.copy(pra[:, 1:2], v1[:1, 0:1])
        nc.scalar.copy(pra[:, 2:3], v2[:1, 0:1])

        st = pst.tile([C, 2 * N], F32)
        nc.vector.memset(st[:], 0.0)

        h_prev = None
        for ci in range(NCHUNK):
            t0 = ci * Tc
            xt = px.tile([C, Tc], F32, tag="xt")
            nc.sync.dma_start(xt[:], x_in[:, t0:t0 + Tc])

            sq = pt0.tile([C, Tc], F32, tag="t0")
            nc.scalar.activation(sq[:], xt[:], AF.Square)
            m_ = psmall.tile([1, Tc], F32, tag="m")
            mm_ = psmall.tile([1, Tc], F32, tag="mm")
            var_ = psmall.tile([1, Tc], F32, tag="var")
            for si in range(NSUB):
                o = si * SUB
                s1 = qh.tile([1, SUB], F32, tag="a")
                _mm(nc, s1[:], ones_col[:], xt[:, o:o + SUB], False)
                nc.vector.tensor_scalar_mul(m_[:, o:o + SUB], s1[:], 1.0 / C)
                s2 = qh.tile([1, SUB], F32, tag="a")
                _mm(nc, s2[:], ones_col[:], sq[:, o:o + SUB], False)
                nc.vector.tensor_tensor(mm_[:, o:o + SUB], m_[:, o:o + SUB],
                                        m_[:, o:o + SUB], op=OP.mult)
                nc.vector.scalar_tensor_tensor(
                    var_[:, o:o + SUB], s2[:], 1.0 / C, mm_[:, o:o + SUB],
                    op0=OP.mult, op1=OP.subtract)
            lnv = psmall.tile([1, Tc], F32, tag="lnv")
            nc.scalar.activation(lnv[:], var_[:], AF.Ln, bias=epsc[:, 0:1])
            rstd = psmall.tile([1, Tc], F32, tag="rstd")
            nc.scalar.activation(rstd[:], lnv[:], AF.Exp, scale=-0.5)

            xn = pt1.tile([C, Tc], F32, tag="t1")
            for si in range(NSUB):
                o = si * SUB
                mb = qh.tile([C, SUB], F32, tag="a")
                _mm(nc, mb[:], ones_row[:], m_[:1, o:o + SUB], False)
                nc.vector.tensor_tensor(xn[:, o:o + SUB], xt[:, o:o + SUB],
                                        mb[:], op=OP.subtract)
                rb = qh.tile([C, SUB], F32, tag="a")
                _mm(nc, rb[:], ones_row[:], rstd[:1, o:o + SUB], False)
                nc.vector.tensor_tensor(xn[:, o:o + SUB], xn[:, o:o + SUB],
                                        rb[:], op=OP.mult)
            ln_t = pt1.tile([C, Tc], F32, tag="t1")
            nc.scalar.activation(ln_t[:], xn[:], AF.Identity,
                                 bias=v1[:, 1:2], scale=v1[:, 0:1])

            lsq = pt0.tile([C, Tc], F32, tag="t0")
            nc.scalar.activation(lsq[:], ln_t[:], AF.Square)
            s3s = psmall.tile([1, Tc], F32, tag="s3s")
            for si in range(NSUB):
                o = si * SUB
                s3 = qh.tile([1, SUB], F32, tag="a")
                _mm(nc, s3[:], ones_col[:], lsq[:, o:o + SUB], False)
                nc.vector.tensor_scalar_mul(s3s[:, o:o + SUB], s3[:], 1.0)
            lnr = psmall.tile([1, Tc], F32, tag="lnr")
            nc.scalar.activation(lnr[:], s3s[:], AF.Ln, scale=1.0 / C,
                                 bias=epsc[:, 0:1])
            rr = psmall.tile([1, Tc], F32, tag="rr")
            nc.scalar.activation(rr[:], lnr[:], AF.Exp, scale=-0.5)

            h_t = ph.tile([C, TC3], F32, tag="h")
            if ci == 0:
                nc.vector.memset(h_t[:, 0:3], 0.0)
            else:
                nc.vector.tensor_copy(h_t[:, 0:3], h_prev[:, Tc:Tc + 3])
            nc.vector.tensor_copy(prv[:, 3:4], ln_t[:1, 0:1])
            for si in range(NSUB):
                o = si * SUB
                rrb = qh.tile([C, SUB], F32, tag="a")
                _mm(nc, rrb[:], ones_row[:], rr[:1, o:o + SUB], False)
                nc.vector.scalar_tensor_tensor(
                    h_t[:, 3 + o:3 + o + SUB], ln_t[:, o:o + SUB], v1[:, 2:3],
                    rrb[:], op0=OP.mult, op1=OP.mult)
            h_prev = h_t

            xc_h = [pxc.tile([C, Tc], F32, tag=f"xc{hf}", name=f"xc{hf}")
                    for hf in range(2)]
            g_h = [pg.tile([C, Tc], F32, tag=f"g{hf}", name=f"g{hf}")
                   for hf in range(2)]
            for hf in range(2):
                for si in range(NSUB):
                    o = si * SUB
                    ps = qh.tile([C, SUB], F32, tag="a")
                    for k in range(K):
                        _mm(nc, ps[:], w_in[:, k * DI + hf * C:k * DI + hf * C + C],
                            h_t[:, o + k:o + k + SUB], False,
                            start=(k == 0), stop=(k == K - 1))
                    # silu(p+cb) = (p+cb) * 1/(1+exp(-(p+cb)))  (no Silu table
                    # set: >16 ACT set switches break walrus lower_act)
                    e1 = psp.tile([C, SUB], F32, tag="sg1")
                    nc.scalar.activation(e1[:], ps[:], AF.Exp, scale=-1.0,
                                         bias=v2[:, 6 + hf:7 + hf])
                    nc.vector.tensor_scalar_add(e1[:], e1[:], 1.0)
                    r1 = psp.tile([C, SUB], F32, tag="sg2")
                    nc.vector.reciprocal(r1[:], e1[:])
                    nc.vector.scalar_tensor_tensor(
                        xc_h[hf][:, o:o + SUB], ps[:], v2[:, hf:hf + 1], r1[:],
                        op0=OP.add, op1=OP.mult)
                    ps2 = qh.tile([C, SUB], F32, tag="a")
                    _mm(nc, ps2[:], w_z[:, hf * C:hf * C + C],
                        h_t[:, o + 3:o + 3 + SUB], False)
                    e2 = psp.tile([C, SUB], F32, tag="sg1")
                    nc.scalar.activation(e2[:], ps2[:], AF.Exp, scale=-1.0)
                    nc.vector.tensor_scalar_add(e2[:], e2[:], 1.0)
                    r2 = psp.tile([C, SUB], F32, tag="sg2")
                    nc.vector.reciprocal(r2[:], e2[:])
                    nc.vector.tensor_tensor(g_h[hf][:, o:o + SUB], ps2[:], r2[:],
                                            op=OP.mult)

            # dbl split into three partition-0-aligned tiles (PE rhs
            # base-partition constraint)
            dproj = pdbl.tile([R, Tc], F32, tag="dproj")
            bhi = pdbl.tile([N, Tc], BF16, tag="bhi")
            blo = pdbl.tile([N, Tc], BF16, tag="blo")
            chi = pdbl.tile([N, Tc], BF16, tag="chi")
            clo = pdbl.tile([N, Tc], BF16, tag="clo")
            for si in range(NSUB):
                o = si * SUB
                for lo, sz, hi_t, lo_t in ((R, N, bhi, blo), (R + N, N, chi, clo)):
                    ps = qh.tile([sz, SUB], F32, tag="a", name="psdbl")
                    for hf in range(2):
                        _mm(nc, ps[:], w_xp[:, hf * 38 + lo:hf * 38 + lo + sz],
                            xc_h[hf][:, o:o + SUB], MM_DBL_R,
                            start=(hf == 0), stop=(hf == 1))
                    nc.scalar.copy(hi_t[:, o:o + SUB], ps[:])
                    nc.vector.tensor_tensor(lo_t[:, o:o + SUB], ps[:],
                                            hi_t[:, o:o + SUB], op=OP.subtract)
                ps = qh.tile([R, SUB], F32, tag="a", name="psdp")
                for hf in range(2):
                    _mm(nc, ps[:], w_xp[:, hf * 38:hf * 38 + R],
                        xc_h[hf][:, o:o + SUB], MM_DBL_R,
                        start=(hf == 0), stop=(hf == 1))
                nc.scalar.copy(dproj[:, o:o + SUB], ps[:])

            dt_h = [pdt.tile([C, Tc], F32, tag=f"dt{hf}", name=f"dt{hf}")
                    for hf in range(2)]
            for hf in range(2):
                for si in range(NSUB):
                    o = si * SUB
                    ps = qh.tile([C, SUB], F32, tag="a")
                    _mm(nc, ps[:], w_dt[:, hf * C:hf * C + C],
                        dproj[0:R, o:o + SUB], MM_DT_R)
                    ab = psp.tile([C, SUB], F32, tag="spa")
                    nc.scalar.activation(ab[:], ps[:], AF.Abs,
                                         bias=v2[:, 2 + hf:3 + hf])
                    ex = psp.tile([C, SUB], F32, tag="spe")
                    nc.scalar.activation(ex[:], ab[:], AF.Exp, scale=-1.0)
                    lg = psp.tile([C, SUB], F32, tag="spl")
                    nc.scalar.activation(lg[:], ex[:], AF.Ln, bias=1.0)
                    rl = psp.tile([C, SUB], F32, tag="spr")
                    nc.scalar.activation(rl[:], ps[:], AF.Relu,
                                         bias=v2[:, 2 + hf:3 + hf])
                    nc.vector.tensor_tensor(dt_h[hf][:, o:o + SUB], lg[:], rl[:],
                                            op=OP.add)

            dtx_h = []
            for hf in range(2):
                dx = pdtx.tile([C, Tc], F32, tag=f"dtx{hf}", name=f"dtx{hf}")
                nc.vector.tensor_tensor(dx[:], dt_h[hf][:], xc_h[hf][:],
                                        op=OP.mult)
                dtx_h.append(dx)

            yps = [[qy.tile([C, SUB], F32, tag=f"y{hf}_{si}", name=f"y{hf}_{si}")
                    for si in range(NSUB)] for hf in range(2)]
            for n in range(N):
                for hf in range(2):
                    idx = n * 2 + hf
                    dA = psc.tile([C, Tc], F32, tag="dA")
                    nc.scalar.activation(dA[:], dt_h[hf][:], AF.Exp,
                                         scale=w_a[:, idx:idx + 1])
                    bt = psc.tile([C, Tc], F32, tag="bt")
                    for si in range(NSUB):
                        o = si * SUB
                        bps = qbc.tile([C, SUB], F32, tag="bc")
                        nc.tensor.matmul(bps[:], sel[:, n * C:(n + 1) * C],
                                         bhi[:, o:o + SUB], start=True, stop=False)
                        nc.tensor.matmul(bps[:], sel[:, n * C:(n + 1) * C],
                                         blo[:, o:o + SUB], start=False, stop=True)
                        nc.vector.scalar_tensor_tensor(
                            bt[:, o:o + SUB], dtx_h[hf][:, o:o + SUB], 1.0,
                            bps[:], op0=OP.mult, op1=OP.mult)
                    ht = phh.tile([C, Tc], F32, tag="ht")
                    nc.vector.tensor_tensor_scan(
                        ht[:], dA[:], bt[:], st[:, idx:idx + 1],
                        op0=OP.mult, op1=OP.add)
                    nc.vector.tensor_copy(st[:, idx:idx + 1], ht[:, Tc - 1:Tc])
                    for si in range(NSUB):
                        o = si * SUB
                        cps = qbc.tile([C, SUB], F32, tag="bc")
                        nc.tensor.matmul(cps[:], sel[:, n * C:(n + 1) * C],
                                         chi[:, o:o + SUB], start=True, stop=False)
                        nc.tensor.matmul(cps[:], sel[:, n * C:(n + 1) * C],
                                         clo[:, o:o + SUB], start=False, stop=True)
                        hc = phc.tile([C, SUB], F32, tag="hc")
                        nc.vector.scalar_tensor_tensor(
                            hc[:], ht[:, o:o + SUB], 1.0, cps[:],
                            op0=OP.mult, op1=OP.mult)
                        _mm(nc, yps[hf][si][:], eye[:], hc[:], False,
                            start=(n == 0), stop=(n == N - 1),
                            skip_group_check=True)

            yg_h = []
            for hf in range(2):
                ya = ptail.tile([C, Tc], F32, tag=f"ya{hf}", name=f"ya{hf}")
                for si in range(NSUB):
                    o = si * SUB
                    nc.vector.scalar_tensor_tensor(
                        ya[:, o:o + SUB], xc_h[hf][:, o:o + SUB],
                        v2[:, 4 + hf:5 + hf], yps[hf][si][:],
                        op0=OP.mult, op1=OP.add)
                yg = ptail.tile([C, Tc], F32, tag=f"yg{hf}", name=f"yg{hf}")
                nc.vector.tensor_tensor(yg[:], ya[:], g_h[hf][:], op=OP.mult)
                yg_h.append(yg)
            for si in range(NSUB):
                o = si * SUB
                pso = qh.tile([C, SUB], F32, tag="a")
                for hf in range(2):
                    _mm(nc, pso[:], w_out[:, hf * C:hf * C + C],
                        yg_h[hf][:, o:o + SUB], MM_OUT_R,
                        start=(hf == 0), stop=(hf == 1))
                ot = ptail.tile([C, SUB], F32, tag="ot")
                nc.vector.scalar_tensor_tensor(
                    ot[:], xt[:, o:o + SUB], gate_c[:, 0:1], pso[:],
                    op0=OP.mult, op1=OP.add)
                nc.sync.dma_start(p_out[:, t0 + o:t0 + o + SUB], ot[:])

    nc.compile()
    return nc


# ---------------------------------------------------------------- host side

def shuffle_channels(x):
    c = x.shape[0]
    return x.reshape(2, c // 2, -1).transpose(1, 0, 2).reshape(c, -1)


def pack_core_inputs(i, dr, b, x1, x2, inw, convw, convb, xpw, dtw, dtb,
                     Alog, Dp, outw, rmsw, lnw, lnb):
    xs = x1 if i == 0 else x2
    x = shuffle_channels(np.asarray(xs[b], np.float32))
    if dr == 1:
        x = x[:, ::-1]
    x = np.ascontiguousarray(x)

    inw_i = np.asarray(inw[i], np.float32)
    cw = np.asarray(convw[i, dr], np.float32)
    cb = np.asarray(convb[i, dr], np.float32)
    xp = np.asarray(xpw[i, dr], np.float32)
    dw = np.asarray(dtw[i, dr], np.float32)
    db = np.asarray(dtb[i, dr], np.float32)
    Av = -np.exp(np.asarray(Alog[i, dr], np.float32))
    Dv = np.asarray(Dp[i, dr], np.float32)
    ow = np.asarray(outw[i], np.float32)

    wIN = np.empty((C, K * DI), np.float32)
    inw_x = inw_i[:DI]
    for k in range(K):
        wIN[:, k * DI:(k + 1) * DI] = (cw[:, k][:, None] * inw_x).T
    wZ = np.ascontiguousarray(inw_i[DI:].T)
    wXP = np.empty((C, 2 * 38), np.float32)
    for hf in range(2):
        wXP[:, hf * 38:(hf + 1) * 38] = xp[:, hf * C:(hf + 1) * C].T
    wDT = np.ascontiguousarray(dw.T)
    wA = np.empty((C, 2 * N), np.float32)
    for nn in range(N):
        for hf in range(2):
            wA[:, nn * 2 + hf] = Av[hf * C:(hf + 1) * C, nn]
    wOUT = np.empty((C, 2 * C), np.float32)
    for hf in range(2):
        wOUT[:, hf * C:(hf + 1) * C] = 0.5 * ow[:, hf * C:(hf + 1) * C].T
    vec2 = np.ascontiguousarray(
        np.stack([cb[:C], cb[C:], db[:C], db[C:], Dv[:C], Dv[C:],
                  -cb[:C], -cb[C:]], axis=1), dtype=np.float32)
    vec1 = np.ascontiguousarray(
        np.stack([np.asarray(lnw[i], np.float32),
                  np.asarray(lnb[i], np.float32),
                  np.asarray(rmsw[i], np.float32)], axis=1), dtype=np.float32)
    gate = np.array([[1.0 if dr == 0 else 0.0]], np.float32)
    eye = np.eye(C, dtype=np.float32)
    try:
        import ml_dtypes
        _bf16 = ml_dtypes.bfloat16
    except Exception:
        import jax.numpy as _jnp
        _bf16 = _jnp.bfloat16
    sel = np.zeros((N, N * C), _bf16)
    for nn in range(N):
        sel[nn, nn * C:(nn + 1) * C] = 1.0
    return {
        "x": x, "wIN": wIN, "wZ": wZ, "wXP": wXP, "wDT": wDT, "wA": wA,
        "wOUT": wOUT, "vec2": vec2, "vec1": vec1, "gate": gate, "eye": eye,
        "sel": sel,
    }


def make_in_maps(inputs):
    args = dict(
        x1=np.asarray(inputs["x1"], np.float32),
        x2=np.asarray(inputs["x2"], np.float32),
        inw=np.asarray(inputs["inw"], np.float32),
        convw=np.asarray(inputs["convw"], np.float32),
        convb=np.asarray(inputs["convb"], np.float32),
        xpw=np.asarray(inputs["xpw"], np.float32),
        dtw=np.asarray(inputs["dtw"], np.float32),
        dtb=np.asarray(inputs["dtb"], np.float32),
        Alog=np.asarray(inputs["Alog"], np.float32),
        Dp=np.asarray(inputs["Dp"], np.float32),
        outw=np.asarray(inputs["outw"], np.float32),
        rmsw=np.asarray(inputs["rmsw"], np.float32),
        lnw=np.asarray(inputs["lnw"], np.float32),
        lnb=np.asarray(inputs["lnb"], np.float32),
    )
    in_maps, core_meta = [], []
    for i in range(2):
        for dr in range(2):
            for b in range(2):
                in_maps.append(pack_core_inputs(i, dr, b, **args))
                core_meta.append((i, dr, b))
    return in_maps, core_meta


def assemble_outputs(results, core_meta):
    B = 2
    outs = []
    for i in range(2):
        acc = np.zeros((B, C, L_FULL), np.float32)
        for (ii, dr, b), res in zip(core_meta, results):
            if ii != i:
                continue
            p = res["p"]
            if dr == 1:
                p = p[:, ::-1]
            acc[b] += p
        outs.append(acc.reshape(B, C, HH, WW))
    return tuple(outs)


# ------------------------------------------------------------- PJRT executor

class _BassExec:
    def __init__(self, nc, n_cores):
        import jax
        from jax.sharding import Mesh, PartitionSpec
        from jax.experimental.shard_map import shard_map
        from concourse.bass2jax import (_bass_exec_p, install_neuronx_cc_hook,
                                        partition_id_tensor)
        install_neuronx_cc_hook()
        self.jax = jax
        self.n_cores = n_cores
        partition_name = (nc.partition_id_tensor.name
                          if nc.partition_id_tensor else None)
        in_names, out_names, out_avals, zero_outs = [], [], [], []
        for alloc in nc.m.functions[0].allocations:
            if not isinstance(alloc, mybir.MemoryLocationSet):
                continue
            name = alloc.memorylocations[0].name
            if alloc.kind == "ExternalInput":
                if name != partition_name:
                    in_names.append(name)
            elif alloc.kind == "ExternalOutput":
                shape = tuple(alloc.tensor_shape)
                dtype = mybir.dt.np(alloc.dtype)
                out_names.append(name)
                out_avals.append(jax.core.ShapedArray(shape, dtype))
                zero_outs.append(np.zeros(shape, dtype))
        self.in_names, self.out_names = in_names, out_names
        self.out_avals, self.zero_outs = out_avals, zero_outs
        n_params, n_outs = len(in_names), len(out_avals)
        bind_names = in_names + out_names + ([partition_name] if partition_name
                                             else [])

        def _body(*args):
            operands = list(args)
            if partition_name is not None:
                operands.append(partition_id_tensor())
            outs = _bass_exec_p.bind(
                *operands,
                out_avals=tuple(out_avals),
                in_names=tuple(bind_names),
                out_names=tuple(out_names),
                lowering_input_output_aliases=(),
                sim_require_finite=True,
                sim_require_nnan=True,
                nc=nc,
            )
            return tuple(outs)

        devices = jax.devices()[:n_cores]
        self.mesh = Mesh(np.asarray(devices), ("core",))
        in_specs = (PartitionSpec("core"),) * (n_params + n_outs)
        out_specs = (PartitionSpec("core"),) * n_outs
        self.fn = jax.jit(
            shard_map(_body, mesh=self.mesh, in_specs=in_specs,
                      out_specs=out_specs, check_rep=False),
            keep_unused=True)

    def prep(self, in_maps):
        from jax.sharding import NamedSharding, PartitionSpec
        concat_in = [
            np.concatenate([np.asarray(in_maps[c][n])
                            for c in range(self.n_cores)], axis=0)
            for n in self.in_names
        ]
        concat_zero = [
            np.zeros((self.n_cores * z.shape[0], *z.shape[1:]), z.dtype)
            for z in self.zero_outs
        ]
        sh = NamedSharding(self.mesh, PartitionSpec("core"))
        return [self.jax.device_put(a, sh) for a in concat_in + concat_zero]

    def run(self, args):
        outs = self.fn(*args)
        self.jax.block_until_ready(outs)
        return outs

    def results(self, outs):
        res = []
        for c in range(self.n_cores):
            m = {}
            for i, name in enumerate(self.out_names):
                a = np.asarray(outs[i])
                a = a.reshape(self.n_cores, *self.out_avals[i].shape)[c]
                m[name] = a
            res.append(m)
        return res


_CACHE = {}


def _get_exec():
    if "ex" not in _CACHE:
        nc = build_program()
        _CACHE["ex"] = _BassExec(nc, 8)
    return _CACHE["ex"]


def kernel(**inputs):
    H = int(inputs.get("H", HH))
    W = int(inputs.get("W", WW))
    assert H == HH and W == WW, (H, W)
    in_maps, core_meta = make_in_maps(inputs)
    ex = _get_exec()
    args = ex.prep(in_maps)
    outs = ex.run(args)
    res = ex.results(outs)
    return assemble_outputs(res, core_meta)

